# revision 77
# baseline (speedup 1.0000x reference)
"""CaptionNet Trainium2 kernel (8-core SPMD, data-parallel over batch).

Per core (batch shard Bc=32): attention-LSTM recurrence fully on-chip in a
feature-on-partition / batch-on-free layout, bf16 matmul operands with fp32
PSUM accumulation.

v3 design (vs v2):
- Einsum uses M=1 stationaries (one expz column per sample) writing ctx rows
  directly into a single PSUM bank at partition 32*strip+slot; the 8
  SBUF->SBUF gather DMAs and blk staging are gone. One DVE copy + 16 narrow
  PE transposes produce ctxT.
- zx/cx x-projections are computed inline per step into PSUM (no precompute,
  no DVE adds); all biases enter PSUM as rank-1 matmuls; activations read
  PSUM directly.
- The exp chain is split per f-tile and sigmoid(-z) is issued first, so the
  f0 einsum streams overlap the f1 exp computation, and gamma + the
  h-dependent half of the LSTM gate matmuls fill the PE during the exp chain.
- LSTM gate layout is permuted to (g,i,f,o) so the nonlinearity is 2 scalar
  instructions (tanh of g, sigmoid of i|f|o).
- gamma*(1/Z) is pre-merged off-chain -> single post-transpose multiply.
- Vocab filler slots sit at the three real PE stall points (post-transpose,
  post-comb, LSTM tail); dummy matmuls keep the PE p-state high in steps 0-3.
- Vocab weight streaming is batched (one DMA per v-chunk) and scheduled
  just-in-time per m-chunk window.
"""

import numpy as np
import ml_dtypes

import concourse.bass as bass
import concourse.tile as tile
import concourse.mybir as mybir

BF16 = mybir.dt.bfloat16
F32 = mybir.dt.float32
AF = mybir.ActivationFunctionType
OP = mybir.AluOpType

# Problem constants (full size)
B_FULL, T_FULL, H, WV, F, C, V_FULL = 256, 20, 512, 301, 196, 512, 9871
N_CORES = 8
F_HI = 128
F_LO = F - F_HI  # 68

VCHUNK = 512
N_VRES = 11  # resident v-chunks (first N_VRES*512 vocab columns stay in SBUF)


def _tiles(total, step=128):
    return [(i, min(step, total - i)) for i in range(0, total, step)]


def build_program(Bc=32, T=20, V=V_FULL, stage=99, dbg_t=-1, for_sim=False):
    TB = Bc * T
    NG = Bc // 8  # einsum strips (4), each handles 8 samples
    nc = bass.Bass()
    if dbg_t >= 0:
        dbg_ez_d = nc.dram_tensor("dbg_ez", [F, Bc], BF16, kind="ExternalOutput")
        dbg_blk_d = nc.dram_tensor("dbg_blk", [128, 8, C], BF16, kind="ExternalOutput")
        dbg_ctxg_d = nc.dram_tensor("dbg_ctxg", [128, 4, Bc], BF16, kind="ExternalOutput")
        dbg_inp_d = nc.dram_tensor("dbg_inp", [128, 3, Bc], BF16, kind="ExternalOutput")
        dbg_h_d = nc.dram_tensor("dbg_h", [128, 4, Bc * T], BF16, kind="ExternalOutput")
        dbg_cb_d = nc.dram_tensor("dbg_cb", [128, 3, Bc], F32, kind="ExternalOutput")

    # ---------------- DRAM I/O (per-core) ----------------
    encT_d = nc.dram_tensor("encT", [Bc, F, C], BF16, kind="ExternalInput")
    xT_d = nc.dram_tensor("xT", [WV, TB], BF16, kind="ExternalInput")
    AxT_d = nc.dram_tensor("AxT", [WV, F], BF16, kind="ExternalInput")
    AhT_d = nc.dram_tensor("AhT", [H, F], BF16, kind="ExternalInput")
    WxT_d = nc.dram_tensor("WxT", [WV, WV], BF16, kind="ExternalInput")
    WcT_d = nc.dram_tensor("WcT", [C, WV], BF16, kind="ExternalInput")
    gateTa_d = nc.dram_tensor("gateTa", [H + 1, C], BF16, kind="ExternalInput")
    WihT_d = nc.dram_tensor("WihT", [WV, 4 * H], BF16, kind="ExternalInput")
    WhhT_d = nc.dram_tensor("WhhT", [H, 4 * H], BF16, kind="ExternalInput")
    vWT_d = nc.dram_tensor("vWT", [H, V], BF16, kind="ExternalInput")
    attnb_d = nc.dram_tensor("attn_br", [1, F], BF16, kind="ExternalInput")
    combb_d = nc.dram_tensor("comb_br", [1, WV], BF16, kind="ExternalInput")
    lstmb_d = nc.dram_tensor("lstm_br", [1, 4 * H], BF16, kind="ExternalInput")
    eye_d = nc.dram_tensor("eye", [Bc, Bc], BF16, kind="ExternalInput")
    onesc_d = nc.dram_tensor("ones_col", [F, 1], BF16, kind="ExternalInput")
    onesr_d = nc.dram_tensor("ones_row", [1, TB], BF16, kind="ExternalInput")
    onesf_d = nc.dram_tensor("ones_f32", [1, 128], F32, kind="ExternalInput")
    out_d = nc.dram_tensor("out", [TB, V], BF16, kind="ExternalOutput")

    wv_t = _tiles(WV)   # [(0,128),(128,128),(256,45)]
    h_t = _tiles(H)     # 4 x 128
    f_t = [(0, F_HI), (F_HI, F_LO)]
    NWV, NH, NF = len(wv_t), len(h_t), len(f_t)
    n_mv = _tiles(TB)   # vocab m-chunks along T*Bc (5 x 128)
    v_ch = _tiles(V, VCHUNK)  # vocab n-chunks (20)
    NVC = len(v_ch)

    with tile.TileContext(nc) as tc:
        with (
            tc.tile_pool(name="w", bufs=1) as wp,
            tc.tile_pool(name="act", bufs=2) as ap,
            tc.tile_pool(name="big", bufs=1) as bp,
            tc.tile_pool(name="st", bufs=2) as st,
            tc.tile_pool(name="vs", bufs=4) as vsp,
            tc.tile_pool(name="vo", bufs=2) as vp,
            tc.tile_pool(name="psE", bufs=2, space="PSUM") as psE,
            tc.tile_pool(name="psV", bufs=2, space="PSUM") as psV,
            tc.tile_pool(name="psS", bufs=1, space="PSUM") as psS,
        ):

            # ---------------- resident loads ----------------
            def load_ktiles(dram, ktiles, ncols, dt, name, q=nc.sync):
                out = []
                for ki, (k0, ks) in enumerate(ktiles):
                    tl = wp.tile([ks, ncols], dt, tag=f"{name}{ki}", name=f"{name}{ki}")
                    q.dma_start(tl[:], dram[k0 : k0 + ks, :])
                    out.append(tl)
                return out

            # order matters: earliest-needed first
            xT_sb = load_ktiles(xT_d, wv_t, TB, BF16, "xT")
            attnb_sb = wp.tile([1, F], BF16, tag="attnbr", name="attnbr")
            nc.sync.dma_start(attnb_sb[:], attnb_d[:])
            ones1 = wp.tile([1, TB], BF16, tag="onesr", name="onesr")
            nc.sync.dma_start(ones1[:], onesr_d[:])
            AxT_sb = load_ktiles(AxT_d, wv_t, F, BF16, "AxT")
            AhT_sb = load_ktiles(AhT_d, h_t, F, BF16, "AhT", q=nc.gpsimd)
            ones_f = load_ktiles(onesc_d, f_t, 1, BF16, "ones", q=nc.gpsimd)
            onesf32 = wp.tile([1, 128], F32, tag="onesf32", name="onesf32")
            nc.gpsimd.dma_start(onesf32[:], onesf_d[:])
            eye_sb = wp.tile([Bc, Bc], BF16, tag="eye", name="eye")
            nc.gpsimd.dma_start(eye_sb[:], eye_d[:])
            gateT_sb = load_ktiles(gateTa_d, h_t, C, BF16, "gateT", q=nc.gpsimd)
            gateB_sb = wp.tile([1, C], BF16, tag="gateB", name="gateB")
            nc.gpsimd.dma_start(gateB_sb[:], gateTa_d[H : H + 1, :])

            # encoding per f-tile x strip; strip s round r holds sample 8s+r
            encT_r = encT_d.rearrange("b f c -> f b c")
            enc_sb = [[None] * NG for _ in range(NF)]
            for fi, (f0, fs) in enumerate(f_t):
                for g in range(NG):
                    e = wp.tile([fs, 8, C], BF16, tag=f"enc{fi}_{g}", name=f"enc{fi}_{g}")
                    for hh in range(2):
                        nc.sync.dma_start(
                            e[:, 4 * hh : 4 * hh + 4, :],
                            encT_r[f0 : f0 + fs, 8 * g + 4 * hh : 8 * g + 4 * hh + 4, :],
                        )
                    enc_sb[fi][g] = e

            WxT_sb = load_ktiles(WxT_d, wv_t, WV, BF16, "WxT", q=nc.scalar)
            combb_sb = wp.tile([1, WV], BF16, tag="combbr", name="combbr")
            nc.scalar.dma_start(combb_sb[:], combb_d[:])
            WcT_sb = load_ktiles(WcT_d, h_t, WV, BF16, "WcT", q=nc.scalar)
            WihT_sb = load_ktiles(WihT_d, wv_t, 4 * H, BF16, "WihT", q=nc.gpsimd)
            WhhT_sb = load_ktiles(WhhT_d, h_t, 4 * H, BF16, "WhhT", q=nc.gpsimd)
            lstmb_sb = wp.tile([1, 4 * H], BF16, tag="lstmb", name="lstmb")
            nc.gpsimd.dma_start(lstmb_sb[:], lstmb_d[:])

            # resident part of the vocab weights [128, NH, N_VRES*VCHUNK]
            VRES = N_VRES * VCHUNK
            vWT_r = vWT_d.rearrange("(a p) v -> p a v", p=128)
            vres = wp.tile([128, NH, VRES], BF16, tag="vres", name="vres")
            for ki in range(NH):
                nc.sync.dma_start(
                    vres[:, ki, :], vWT_d[128 * ki : 128 * ki + 128, 0:VRES]
                )

            h_all = bp.tile([128, NH, TB], BF16, tag="h_all", name="h_all")
            # per-m-chunk snapshots of h for the vocab projection (breaks the
            # false dependency of vocab matmuls on later h writes)
            hv = [
                bp.tile([128, NH, ms], BF16, tag=f"hv{j}", name=f"hv{j}")
                for j, (m0, ms) in enumerate(n_mv)
            ]
            # staging for the 8 einsum rounds (copied out of PSUM per round)
            blk = bp.tile([128, 8, C], BF16, tag="blk", name="blk")

            if stage < 1:
                return nc

            # ---------------- vocab projection slots ----------------
            def vocab_slot(j, vw_ap, n0, nn, eng):
                m0, ms = n_mv[j]
                vps = psV.tile([128, VCHUNK], F32, tag="voc", name="voc")
                for ki in range(NH):
                    nc.tensor.matmul(
                        vps[0:ms, 0:nn],
                        hv[j][:, ki, 0:ms],
                        vw_ap[:, ki, 0:nn],
                        start=(ki == 0),
                        stop=(ki == NH - 1),
                    )
                vo = vp.tile([128, VCHUNK], BF16, tag="vout", name="vout")
                if eng == 0:
                    nc.scalar.activation(vo[0:ms, 0:nn], vps[0:ms, 0:nn], AF.Copy)
                else:
                    nc.vector.tensor_copy(vo[0:ms, 0:nn], vps[0:ms, 0:nn])
                nc.sync.dma_start(out_d[m0 : m0 + ms, n0 : n0 + nn], vo[0:ms, 0:nn])

            # vocab work schedule: m-chunk j's 20 v-chunks are emitted in its
            # window of steps 4j+4 .. 4j+7 (5 per step); streamed v-chunks are
            # DMA'd just-in-time, 2 loads per step in the window.
            vstream = {}  # v -> current sbuf tile (rotating bufs)

            def vload(v):
                n0, nn = v_ch[v]
                vst = vsp.tile([128, NH, VCHUNK], BF16, tag="vs", name="vs")
                nc.sync.dma_start(vst[:, :, 0:nn], vWT_r[:, :, n0 : n0 + nn])
                vstream[v] = vst

            def emit_item(j, v, eng):
                n0, nn = v_ch[v]
                if v < N_VRES:
                    vocab_slot(j, vres[:, :, n0 : n0 + nn], n0, nn, eng)
                else:
                    vocab_slot(j, vstream[v][:, :, :], n0, nn, eng)

            # per-step plans: loads[t] = v-chunks to DMA at step start,
            # items[t] = (j, v) slots for this step's filler points.
            loads = [[] for _ in range(T)]
            items = [[] for _ in range(T)]
            for j in range(len(n_mv) - 1):  # j=4 goes to the epilogue
                w0 = 4 * j + 4
                seq = [(j, v) for v in range(NVC)]
                for k, (jj, v) in enumerate(seq):
                    items[w0 + min(k // 5, 3)].append((jj, v))
                # paced so a load's buffer (vsp bufs=4) is freed by the time
                # the load is dispatched, and data lands before its slot runs
                lpat = [0, 0, 1, 1, 2, 2, 2, 3, 3]
                for k, v in enumerate(range(N_VRES, NVC)):
                    loads[w0 + lpat[k]].append(v)

            # dummy matmul stream (~213ns at full clock) to keep the PE
            # p-state up through chain stalls when no vocab work is ready
            def pad(n=1):
                for _ in range(n):
                    dps = psV.tile([128, VCHUNK], F32, tag="voc", name="dmy")
                    nc.tensor.matmul(
                        dps[:, :], xT_sb[0][:, 0:128], xT_sb[0][:, 0:512],
                        start=True, stop=True,
                    )

            if stage < 2:
                return nc

            # ---------------- recurrence ----------------
            c_prev = None
            h_prev = None
            for t in range(T):
                tc0, tc1 = t * Bc, (t + 1) * Bc
                fill = list(items[t])
                fpos = [0]

                def emit_fill(k, engs):
                    for x in range(k):
                        if fpos[0] < len(fill):
                            j, v = fill[fpos[0]]
                            emit_item(j, v, engs[x % len(engs)])
                            fpos[0] += 1

                def fill_pt(slots, pads, engs):
                    # emit up to `slots` vocab slots; pad the shortfall +
                    # `pads` extra quarter-slot streams to bridge the stall
                    got = 0
                    if t >= 4:
                        while got < slots and fpos[0] < len(fill):
                            j, v = fill[fpos[0]]
                            emit_item(j, v, engs[got % len(engs)])
                            fpos[0] += 1
                            got += 1
                    pad(4 * (slots - got) + pads)

                for v in loads[t]:
                    vload(v)

                # one packed PSUM bank for the step's small tensors:
                # [0:2]=zh(f0,f1), [2:6]=gamT, [6]=Z row then [6:10]=rbc
                sm = psS.tile([128, 10, Bc], F32, tag="small", name="sm")

                # attention z = attn_W @ [x; h] + b, accumulated in PSUM.
                # f0 first so its exp chain starts while f1/gamma/gates-h run.
                expz = []
                for fi, (f0, fs) in enumerate(f_t):
                    for ki, (k0, ks) in enumerate(wv_t):
                        nc.tensor.matmul(
                            sm[0:fs, fi, :],
                            AxT_sb[ki][:, f0 : f0 + fs],
                            xT_sb[ki][:, tc0:tc1],
                            start=(ki == 0),
                            stop=False,
                        )
                    nc.tensor.matmul(
                        sm[0:fs, fi, :],
                        attnb_sb[:, f0 : f0 + fs],
                        ones1[:, 0:Bc],
                        start=False,
                        stop=(t == 0),
                    )
                    if t > 0:
                        for ki in range(NH):
                            nc.tensor.matmul(
                                sm[0:fs, fi, :],
                                AhT_sb[ki][:, f0 : f0 + fs],
                                h_prev[ki],
                                start=False,
                                stop=(ki == NH - 1),
                            )
                    # exp(z) = sigmoid(z)/sigmoid(-z); sigmoid(-z) first so
                    # the DVE reciprocal starts as early as possible.
                    sn = ap.tile([fs, Bc], F32, tag=f"sn{fi}", name=f"sn{fi}")
                    nc.scalar.activation(sn[:], sm[0:fs, fi, :], AF.Sigmoid, scale=-1.0)
                    rn = ap.tile([fs, Bc], F32, tag=f"rn{fi}", name=f"rn{fi}")
                    nc.vector.reciprocal(rn[:], sn[:])
                    sp = ap.tile([fs, Bc], F32, tag=f"sp{fi}", name=f"sp{fi}")
                    nc.scalar.activation(sp[:], sm[0:fs, fi, :], AF.Sigmoid)
                    ez = ap.tile([fs, 1, Bc], BF16, tag=f"expz{fi}", name=f"expz{fi}")
                    nc.gpsimd.tensor_tensor(ez[:, 0, :], sp[:], rn[:], op=OP.mult)
                    expz.append(ez)

                # gamma pre-activation, C-major: gamT[c, b]  (fills exp stall)
                for m in range(NH):
                    if t > 0:
                        for ki in range(NH):
                            nc.tensor.matmul(
                                sm[:, 2 + m, :],
                                gateT_sb[ki][:, 128 * m : 128 * m + 128],
                                h_prev[ki],
                                start=(ki == 0),
                                stop=False,
                            )
                    nc.tensor.matmul(
                        sm[:, 2 + m, :],
                        gateB_sb[:, 128 * m : 128 * m + 128],
                        ones1[:, 0:Bc],
                        start=(t == 0),
                        stop=True,
                    )
                del m
                gamS = ap.tile([128, NH, Bc], BF16, tag="gamS", name="gamS")
                nc.scalar.activation(gamS[:], sm[:, 2:6, :], AF.Sigmoid)

                # LSTM gates psum (filled after relu; one open group per
                # bank at a time, so the whole per-m group is contiguous)
                g_ps = psS.tile([128, 16, Bc], F32, tag="gates", name="gates")

                fill_pt(1, 1, [0])  # P_0: covers the exp chain

                # einsum: ctx[b,:] = sum_f expz[b,f] * enc[b,f,:]
                # Round r, strip s computes sample b=4r+s into psum rows
                # [32s,32s+32) (M=32 broadcast). Each finished round is copied
                # to blk[:, r, :]; a selection-matrix transpose then picks one
                # row per strip and writes ctT[:, jj, 4r:4r+4] directly.
                ctT_ps = psS.tile([128, NH, Bc], BF16, tag="ctT", name="ctT")
                recip = ap.tile([1, 1, Bc], F32, tag="recip", name="recip")
                ctx_sb = ap.tile([Bc, C], BF16, tag="ctx", name="ctx", bufs=1)

                # Z = sum_f expz -> [1, Bc]
                for fi in range(NF):
                    nc.tensor.matmul(
                        sm[0:1, 6, :], ones_f[fi][:], expz[fi][:, 0, :],
                        start=(fi == 0), stop=(fi == NF - 1),
                    )
                nc.vector.reciprocal(recip[:, 0, :], sm[0:1, 6, :])

                def e_round(r):
                    eps = psE.tile([128, C], F32, tag="ein", name="ein")
                    for s in range(NG):
                        b = 8 * s + r
                        for fi in range(NF):
                            nc.tensor.matmul(
                                eps[32 * s : 32 * s + 32, :],
                                expz[fi][:, 0, b : b + 1].broadcast_to(
                                    [f_t[fi][1], 32]
                                ),
                                enc_sb[fi][s][:, r, :],
                                start=(fi == 0),
                                stop=(fi == NF - 1),
                                tile_position=(0, 32 * s),
                            )
                    if r % 2:
                        nc.scalar.activation(blk[:, r, :], eps[:, :], AF.Copy)
                    else:
                        nc.vector.tensor_copy(blk[:, r, :], eps[:, :])

                qs = (nc.gpsimd, nc.sync, nc.gpsimd, nc.scalar)

                def e_gather(hh):
                    # rows 8s+4hh..8s+4hh+4 of ctx from strip s, rounds 4hh..
                    for s in range(NG):
                        qs[s].dma_start(
                            ctx_sb[8 * s + 4 * hh : 8 * s + 4 * hh + 4, :],
                            blk[32 * s : 32 * s + 1, 4 * hh : 4 * hh + 4, :],
                        )

                for r in range(8):
                    e_round(r)
                    if r == 3:
                        e_gather(0)
                    if r == 4:
                        # 1/Z broadcast to all partitions via PE
                        nc.tensor.matmul(
                            sm[:, 6:10, :], onesf32[:],
                            recip[:, :, :].broadcast_to([1, NH, Bc]),
                            start=True, stop=True,
                        )
                        g2 = ap.tile([128, NH, Bc], BF16, tag="g2", name="g2")
                        nc.vector.tensor_tensor(
                            g2[:], gamS[:], sm[:, 6:10, :], op=OP.mult
                        )
                e_gather(1)
                fill_pt(0, 6, [])  # E: covers copy(7) + gather latency
                # transpose ctx -> ctT [c, b] (4 narrow PE transposes)
                for jj in range(NH):
                    nc.tensor.transpose(
                        ctT_ps[:, jj, :],
                        ctx_sb[:, 128 * jj : 128 * jj + 128],
                        eye_sb[:],
                    )

                cb = psE.tile([128, NWV, Bc], F32, tag="cb", name="cb", bufs=1)
                # ctxgT = ctT * sigmoid(gamma)/Z  (g2 computed mid-einsum)
                ctxgT = ap.tile([128, NH, Bc], BF16, tag="ctxgT", name="ctxgT")
                nc.vector.tensor_tensor(ctxgT[:], ctT_ps[:], g2[:], op=OP.mult)

                if t == dbg_t:
                    for fi, (f0, fs) in enumerate(f_t):
                        nc.sync.dma_start(
                            dbg_ez_d[f0 : f0 + fs, :], expz[fi][:, 0, :]
                        )
                    nc.sync.dma_start(dbg_blk_d[:], blk[:])
                    nc.sync.dma_start(dbg_ctxg_d[:], ctxgT[:])

                fill_pt(1, 0, [0])  # P_A: covers g2/ctxgT

                # comb: per-mi contiguous group (x + bias + ctx), relu from
                # PSUM. One group open at a time in the cb bank.
                for mi, (m0, ms) in enumerate(wv_t):
                    for ki, (k0, ks) in enumerate(wv_t):
                        nc.tensor.matmul(
                            cb[0:ms, mi, :],
                            WxT_sb[ki][:, m0 : m0 + ms],
                            xT_sb[ki][:, tc0:tc1],
                            start=(ki == 0),
                            stop=False,
                        )
                    nc.tensor.matmul(
                        cb[0:ms, mi, :],
                        combb_sb[:, m0 : m0 + ms],
                        ones1[:, 0:Bc],
                        start=False,
                        stop=False,
                    )
                    for ki in range(NH):
                        nc.tensor.matmul(
                            cb[0:ms, mi, :],
                            WcT_sb[ki][:, m0 : m0 + ms],
                            ctxgT[:, ki, :],
                            start=False,
                            stop=(ki == NH - 1),
                        )
                if t == dbg_t:
                    cbs = ap.tile([128, NWV, Bc], F32, tag="cbs", name="cbs", bufs=1)
                    for mi, (m0, ms) in enumerate(wv_t):
                        nc.vector.tensor_copy(cbs[0:ms, mi, :], cb[0:ms, mi, :])
                        nc.sync.dma_start(dbg_cb_d[0:ms, mi, :], cbs[0:ms, mi, :])
                inp_bf = ap.tile([128, NWV, Bc], BF16, tag="inp", name="inp")
                nc.scalar.activation(inp_bf[:, 0:2, :], cb[:, 0:2, :], AF.Relu)
                nc.scalar.activation(inp_bf[0:45, 2, :], cb[0:45, 2, :], AF.Relu)

                fill_pt(1, 1, [1])  # P_B: covers the relu

                # LSTM gates: per-m contiguous group (bias + h + inp).
                # permuted gate order: g=0:4, i=4:8, f=8:12, o=12:16
                for m in range(16):
                    nc.tensor.matmul(
                        g_ps[:, m, :],
                        lstmb_sb[:, m * 128 : (m + 1) * 128],
                        ones1[:, 0:Bc],
                        start=True,
                        stop=False,
                    )
                    if t > 0:
                        for ki in range(NH):
                            nc.tensor.matmul(
                                g_ps[:, m, :],
                                WhhT_sb[ki][:, m * 128 : (m + 1) * 128],
                                h_prev[ki],
                                start=False,
                                stop=False,
                            )
                    for ki, (k0, ks) in enumerate(wv_t):
                        nc.tensor.matmul(
                            g_ps[:, m, :],
                            WihT_sb[ki][:, m * 128 : (m + 1) * 128],
                            inp_bf[0:ks, ki, :],
                            start=False,
                            stop=(ki == NWV - 1),
                        )
                del m

                # LSTM nonlinearity: tanh(g), sigmoid(i|f|o)
                tg = ap.tile([128, 4, Bc], F32, tag="tg", name="tg")
                nc.scalar.activation(tg[:], g_ps[:, 0:4, :], AF.Tanh)
                sio = ap.tile([128, 12, Bc], F32, tag="sio", name="sio")
                nc.scalar.activation(sio[:], g_ps[:, 4:16, :], AF.Sigmoid)
                c_new = st.tile([128, 4, Bc], F32, tag="c", name="c")
                if t > 0:
                    ig = ap.tile([128, 4, Bc], F32, tag="ig", name="ig")
                    nc.gpsimd.tensor_tensor(ig[:], tg[:], sio[:, 0:4, :], op=OP.mult)
                    cf = ap.tile([128, 4, Bc], F32, tag="cf", name="cf")
                    nc.vector.tensor_tensor(cf[:], sio[:, 4:8, :], c_prev[:], op=OP.mult)
                    nc.gpsimd.tensor_tensor(c_new[:], ig[:], cf[:], op=OP.add)
                else:
                    nc.gpsimd.tensor_tensor(c_new[:], tg[:], sio[:, 0:4, :], op=OP.mult)
                tanh_c = ap.tile([128, 4, Bc], F32, tag="tanh_c", name="tanh_c")
                nc.scalar.activation(tanh_c[:], c_new[:], AF.Tanh)
                nc.vector.tensor_tensor(
                    h_all[:, :, tc0:tc1], sio[:, 8:12, :], tanh_c[:], op=OP.mult
                )
                if t == dbg_t:
                    nc.sync.dma_start(dbg_inp_d[:], inp_bf[:])

                c_prev = c_new
                h_prev = [h_all[:, k, tc0:tc1] for k in range(NH)]
                if dbg_t >= 0 and t == T - 1:
                    nc.sync.dma_start(dbg_h_d[:], h_all[:])

                # snapshot a finished 128-token chunk of h for the vocab slots
                if (t + 1) % 4 == 0:
                    j = (t + 1) // 4 - 1
                    m0, ms = n_mv[j]
                    nc.gpsimd.tensor_copy(hv[j][:], h_all[:, :, m0 : m0 + ms])

                # P_C: emitted after the tail so the PE runs these matmuls
                # during the tail while the copies queue behind the tail's
                # scalar/vector/gpsimd work instead of ahead of it.
                emit_fill(len(fill) - fpos[0], [1, 0])
                pad(14 if t < 4 else 2)

            if stage < 3:
                return nc

            # ---------------- vocab epilogue: m-chunk j=4 ----------------
            j = len(n_mv) - 1
            for v in range(N_VRES, min(N_VRES + 4, NVC)):
                vload(v)
            for k, v in enumerate(range(N_VRES)):
                emit_item(j, v, k % 2)
            for v in range(N_VRES + 4, NVC):
                vload(v)
            for k, v in enumerate(range(N_VRES, NVC)):
                emit_item(j, v, k % 2)

    if not for_sim:
        _split_multi_waits(nc)
    return nc


def _split_multi_waits(nc):
    """walrus' codegen accepts at most one sync wait per engine instruction
    in this environment; hoist extra waits onto same-engine NoOps placed
    immediately before the owning instruction."""
    for fn in nc.m.functions:
        for bb in fn.blocks:
            insts = bb.instructions
            out = []
            changed = False
            for inst in insts:
                si = inst.sync_info
                if si is not None and len(si.on_wait) > 1:
                    waits = list(si.on_wait)
                    for w in waits[:-1]:
                        out.append(
                            mybir.InstNoOp(
                                name=f"{inst.name}-w{len(out)}",
                                engine=inst.engine,
                                sync_info=mybir.SyncInfo(
                                    on_wait=[w], on_update=[]
                                ),
                            )
                        )
                    inst.sync_info = mybir.SyncInfo(
                        on_wait=[waits[-1]], on_update=list(si.on_update)
                    )
                    changed = True
                out.append(inst)
            if changed:
                bb.instructions = out


# ======================= host side =======================

def _bf16(x):
    return np.ascontiguousarray(np.asarray(x, dtype=ml_dtypes.bfloat16))


def _perm_eye():
    """eye[p, q] = 1 iff p == 32q: picks one row per einsum strip."""
    e = np.zeros((128, 4), np.float32)
    for q in range(4):
        e[32 * q, q] = 1.0
    return e


def _permute_gates(w):
    """PyTorch gate order (i,f,g,o) -> kernel order (g,i,f,o) on dim 0."""
    i, f, g, o = np.split(np.asarray(w, np.float32), 4, axis=0)
    return np.concatenate([g, i, f, o], axis=0)


def prep_shared(inputs, Bc, T, V):
    """Weight-derived in_map entries (replicated across cores)."""
    attn_W = np.asarray(inputs["attn_W"], np.float32)
    comb_W = np.asarray(inputs["comb_W"], np.float32)
    gate_W = np.asarray(inputs["gate_W"], np.float32)
    bsum = _permute_gates(
        np.asarray(inputs["lstm_bih"], np.float32)
        + np.asarray(inputs["lstm_bhh"], np.float32)
    )
    sh = {
        "AxT": _bf16(attn_W[:, :WV].T),
        "AhT": _bf16(attn_W[:, WV:].T),
        "WxT": _bf16(comb_W[:, :WV].T),
        "WcT": _bf16(comb_W[:, WV:].T),
        "gateTa": _bf16(
            np.concatenate(
                [gate_W.T, np.asarray(inputs["gate_b"], np.float32)[None, :]], 0
            )
        ),
        "WihT": _bf16(_permute_gates(np.asarray(inputs["lstm_Wih"])).T),
        "WhhT": _bf16(_permute_gates(np.asarray(inputs["lstm_Whh"])).T),
        "vWT": _bf16(np.asarray(inputs["vocab_W"]).T[:, :V]),
        "attn_br": _bf16(np.asarray(inputs["attn_b"])[None, :]),
        "comb_br": _bf16(np.asarray(inputs["comb_b"])[None, :]),
        "lstm_br": _bf16(bsum[None, :]),
        "eye": np.eye(Bc, dtype=ml_dtypes.bfloat16),
        "ones_col": np.ones((F, 1), dtype=ml_dtypes.bfloat16),
        "ones_row": np.ones((1, T * Bc), dtype=ml_dtypes.bfloat16),
        "ones_f32": np.ones((1, 128), dtype=np.float32),
    }
    return sh


def prep_core(inputs, core, Bc, T, V):
    """Batch-sharded in_map entries for one core."""
    b0, b1 = core * Bc, (core + 1) * Bc
    enc = np.asarray(inputs["encoding"], np.float32)[b0:b1]  # [Bc, C, F]
    wv = np.asarray(inputs["wordvecs"], np.float32)[b0:b1, :T]  # [Bc, T, WV]
    x_shift = np.concatenate(
        [np.zeros((Bc, 1, WV), np.float32), wv[:, :-1, :]], axis=1
    )
    return {
        "encT": _bf16(enc.transpose(0, 2, 1)),  # [Bc, F, C]
        "xT": _bf16(x_shift.transpose(2, 1, 0).reshape(WV, T * Bc)),
    }


_PROG_CACHE = {}
LAST_RESULT = None


def kernel(**inputs):
    global LAST_RESULT
    from concourse.bass_utils import run_bass_kernel_spmd

    Bc, T, V = B_FULL // N_CORES, T_FULL, V_FULL
    key = (Bc, T, V)
    if key not in _PROG_CACHE:
        _PROG_CACHE[key] = build_program(Bc, T, V)
    nc = _PROG_CACHE[key]

    shared = prep_shared(inputs, Bc, T, V)
    in_maps = [dict(shared, **prep_core(inputs, k, Bc, T, V)) for k in range(N_CORES)]
    res = run_bass_kernel_spmd(nc, in_maps, list(range(N_CORES)))
    LAST_RESULT = res

    parts = []
    for r in res.results:
        o = np.asarray(r["out"]).astype(np.float32).reshape(T, Bc, V).transpose(1, 0, 2)
        parts.append(o)
    out = np.concatenate(parts, axis=0)
    out = out + np.asarray(inputs["vocab_b"], np.float32)[None, None, :]
    return np.ascontiguousarray(out.astype(np.float32))
